# revision 3
# baseline (speedup 1.0000x reference)
"""Sharded attention kernel for Trainium2 (8 NeuronCores).

Problem: B=2, T=2048, D=1024, H=16 heads (head dim 64), causal self-attention
with separate Q/K/V projections, key-mask additive bias and post-softmax
query-mask, fp32 reference.

Sharding: data-parallel over the 2 batches x tensor-parallel over 4 head
groups (4 heads each) -> 8 fully independent cores, no collectives.

Per-core plan (all matmuls fp32r, 1 cycle/row on the PE):
  - host pre-transposes x (so the d-contraction sits on partitions) and the
    W^T slices; biases/masks/constants are precomputed host-side too.
  - projections produce qT,kT in [head_dim, T] layout and v in natural
    [tk, head_dim] layout with a ones column appended (softmax denominators
    fall out of the PV matmul for free).
  - scores are computed transposed, S_T[tk, tq] = k.q, one 128-row k-strip x
    512-col q-chunk at a time; softmax needs no reductions at all: bounded
    inputs let us skip the max-subtraction, exp runs on the scalar engine
    with the 1/sqrt(64) scale and key-mask bias fused in, and the denominator
    comes from the ones column of v.
  - causality: blocks entirely above the diagonal are skipped, diagonal
    blocks are exp'd only on their live columns and the 128-wide triangle is
    zeroed with one multiplicative mask; the PV matmul accumulates only live
    columns so the dead region is never touched.
  - ctx^T [65, tq] (64 dims + denominator row) is PE-transposed back in 128
    column blocks, normalized by reciprocal(denominator) * query_mask on the
    vector engine, and streamed out.
"""

import os
import sys
import time

import numpy as np

for _p in ("/opt/trn_rl_repo",):
    if os.path.isdir(_p) and _p not in sys.path:
        sys.path.append(_p)

import concourse.bass as bass  # noqa: E402
import concourse.mybir as mybir  # noqa: E402
import concourse.tile as tile  # noqa: E402
from concourse import bacc  # noqa: E402
from concourse.bass_utils import run_bass_kernel_spmd  # noqa: E402

B, T, D, H = 2, 2048, 1024, 16
HD = D // H          # 64 head dim
NCORES = 8
BG = NCORES // B     # 4 head-groups per batch
HG = H // BG         # 4 heads per core
HDG = HG * HD        # 256 projection cols per core
PB = 128             # partition block
NT = T // PB         # 16 k-strips / t-tiles
QC = 512             # q-chunk width
NCH = T // QC        # 4 q-chunks
KC = D // PB         # 8 contraction chunks
SCALE = 1.0 / (HD ** 0.5)

_CACHE: dict = {}
_STAGE = int(os.environ.get("K_STAGE", "3"))  # 1=proj 2=+attn 3=full (debug)


def _build(mask_future: bool, qk_bias: bool, v_bias: bool, _stage=None):
    f32 = mybir.dt.float32
    f32r = mybir.dt.float32r
    F = mybir.ActivationFunctionType

    nc = bacc.Bacc("TRN2", target_bir_lowering=False, debug=False,
                   num_devices=NCORES)
    xqT = nc.dram_tensor("xqT", [D, T], f32r, kind="ExternalInput").ap()
    xkT = nc.dram_tensor("xkT", [D, T], f32r, kind="ExternalInput").ap()
    wqT = nc.dram_tensor("wqT", [D, HDG], f32r, kind="ExternalInput").ap()
    wkT = nc.dram_tensor("wkT", [D, HDG], f32r, kind="ExternalInput").ap()
    wvT = nc.dram_tensor("wvT", [D, HDG], f32r, kind="ExternalInput").ap()
    kmb = nc.dram_tensor("kmb", [PB, NT], f32, kind="ExternalInput").ap()
    qm = nc.dram_tensor("qm", [PB, NT], f32, kind="ExternalInput").ap()
    ident = nc.dram_tensor("ident", [PB, PB], f32, kind="ExternalInput").ap()
    ones_g = nc.dram_tensor("ones_g", [PB, HG], f32r, kind="ExternalInput").ap()
    causal = None
    if mask_future:
        causal = nc.dram_tensor("causal", [PB, PB], f32r,
                                kind="ExternalInput").ap()
    bq2 = bk2 = bvb = None
    if qk_bias:
        bq2 = nc.dram_tensor("bq2", [PB, 2], f32, kind="ExternalInput").ap()
        bk2 = nc.dram_tensor("bk2", [PB, 2], f32, kind="ExternalInput").ap()
    if v_bias:
        bvb = nc.dram_tensor("bvb", [PB, HDG], f32, kind="ExternalInput").ap()
    out = nc.dram_tensor("out", [T, HDG], f32, kind="ExternalOutput").ap()

    with tile.TileContext(nc) as tc:
        with (
            tc.tile_pool(name="singles", bufs=1) as singles,
            tc.tile_pool(name="xq", bufs=16) as xq_pool,
            tc.tile_pool(name="xk", bufs=16) as xk_pool,
            tc.tile_pool(name="qT", bufs=2) as qT_pool,
            tc.tile_pool(name="kT", bufs=2) as kT_pool,
            tc.tile_pool(name="v", bufs=NT) as v_pool,
            tc.tile_pool(name="pt", bufs=4) as p_pool,
            tc.tile_pool(name="ctxs", bufs=2) as ctxs_pool,
            tc.tile_pool(name="outs", bufs=NT) as outs_pool,
            tc.tile_pool(name="rec", bufs=4) as rec_pool,
            tc.tile_pool(name="pp_a", bufs=4, space="PSUM") as pp_a,
            tc.tile_pool(name="pp_ctx", bufs=2, space="PSUM") as pp_ctx,
            tc.tile_pool(name="pp_t", bufs=2, space="PSUM") as pp_t,
        ):
            # ---- constants / weights
            w_sb = {}
            for name, src in (("q", wqT), ("k", wkT), ("v", wvT)):
                wt = singles.tile([PB, KC, HDG], f32r, tag=f"w{name}")
                nc.sync.dma_start(
                    out=wt, in_=src.rearrange("(c p) n -> p c n", p=PB))
                w_sb[name] = wt
            km_t = singles.tile([PB, NT], f32, tag="km")
            nc.sync.dma_start(out=km_t, in_=kmb)
            qm_t = singles.tile([PB, NT], f32, tag="qm")
            nc.sync.dma_start(out=qm_t, in_=qm)
            id_t = singles.tile([PB, PB], f32, tag="id")
            nc.sync.dma_start(out=id_t, in_=ident)
            cz_t = None
            if mask_future:
                cz_t = singles.tile([PB, PB], f32r, tag="cz")
                nc.sync.dma_start(out=cz_t, in_=causal)
            bq_t = bk_t = bv_t = None
            if qk_bias:
                bq_t = singles.tile([PB, 2], f32, tag="bq")
                nc.sync.dma_start(out=bq_t, in_=bq2)
                bk_t = singles.tile([PB, 2], f32, tag="bk")
                nc.sync.dma_start(out=bk_t, in_=bk2)
            if v_bias:
                bv_t = singles.tile([PB, HDG], f32, tag="bv")
                nc.sync.dma_start(out=bv_t, in_=bvb)

            # ---- projections
            qT_sb = [qT_pool.tile([PB, T], f32r, tag="qT", name=f"qT{i}")
                     for i in range(2)]
            kT_sb = [kT_pool.tile([PB, T], f32r, tag="kT", name=f"kT{i}")
                     for i in range(2)]
            v_sb = [v_pool.tile([PB, HG * (HD + 1)], f32r, tag="v",
                                 name=f"v{i}") for i in range(NT)]

            for ch in range(NCH):
                xq_ts, xk_ts = [], []
                for dc in range(KC):
                    t1 = xq_pool.tile([PB, QC], f32r, tag="xq")
                    nc.sync.dma_start(
                        out=t1,
                        in_=xqT[dc * PB:(dc + 1) * PB, ch * QC:(ch + 1) * QC])
                    xq_ts.append(t1)
                    t2 = xk_pool.tile([PB, QC], f32r, tag="xk")
                    nc.sync.dma_start(
                        out=t2,
                        in_=xkT[dc * PB:(dc + 1) * PB, ch * QC:(ch + 1) * QC])
                    xk_ts.append(t2)
                # q/k projections: out = W^T.T @ x^T -> [head_dim rows, tq]
                for wname, x_ts, dst, bias_t in (("q", xq_ts, qT_sb, bq_t),
                                                 ("k", xk_ts, kT_sb, bk_t)):
                    for ht in range(2):
                        ps = pp_a.tile([PB, QC], f32, tag="mm")
                        for dc in range(KC):
                            nc.tensor.matmul(
                                ps,
                                w_sb[wname][:, dc, ht * PB:(ht + 1) * PB],
                                x_ts[dc],
                                start=(dc == 0), stop=(dc == KC - 1))
                        dslice = dst[ht][:, ch * QC:(ch + 1) * QC]
                        if bias_t is not None:
                            nc.vector.tensor_scalar_add(
                                dslice, ps, bias_t[:, ht:ht + 1])
                        else:
                            nc.vector.tensor_copy(dslice, ps)
                # v projection: natural layout, x^T chunk is the stationary op
                for r in range(QC // PB):
                    tk = ch * (QC // PB) + r
                    ps = pp_a.tile([PB, QC], f32, tag="mm")
                    for dc in range(KC):
                        nc.tensor.matmul(
                            ps[:, 0:HDG],
                            xk_ts[dc][:, r * PB:(r + 1) * PB],
                            w_sb["v"][:, dc, :],
                            start=(dc == 0), stop=(dc == KC - 1))
                    v3 = v_sb[tk].rearrange("p (g c) -> p g c", c=HD + 1)
                    ps3 = ps[:, 0:HDG].rearrange("p (g c) -> p g c", c=HD)
                    if bv_t is not None:
                        nc.vector.tensor_add(
                            v3[:, :, 0:HD], ps3,
                            bv_t.rearrange("p (g c) -> p g c", c=HD))
                    else:
                        nc.vector.tensor_copy(v3[:, :, 0:HD], ps3)
                    nc.sync.dma_start(
                        out=v3[:, :, HD:HD + 1],
                        in_=ones_g.rearrange("p (g o) -> p g o", o=1))

            # ---- attention
            out_sb = [outs_pool.tile([PB, HDG], f32, tag="o",
                                       name=f"os{i}") for i in range(NT)]
            for h in range(HG if _STAGE >= 2 else 0):
                ht, off = h // 2, (h % 2) * HD
                for j in range(NCH):
                    qch = qT_sb[ht][off:off + HD, j * QC:(j + 1) * QC]
                    cps = pp_ctx.tile([HD + 1, QC], f32, tag="ctx")
                    if mask_future:
                        diag0 = j * (QC // PB)
                        order = list(range(diag0, diag0 + QC // PB)) + \
                            list(range(0, diag0))
                    else:
                        order = list(range(NT))
                    for si, i in enumerate(order):
                        c0 = 0
                        if mask_future and i >= diag0:
                            c0 = (i - diag0) * PB
                        sps = pp_a.tile([PB, QC], f32, tag="mm")
                        nc.tensor.matmul(
                            sps[:, c0:QC],
                            kT_sb[ht][off:off + HD, i * PB:(i + 1) * PB],
                            qch[:, c0:QC], start=True, stop=True)
                        pt = p_pool.tile([PB, QC], f32r, tag="p")
                        nc.scalar.activation(
                            out=pt[:, c0:QC], in_=sps[:, c0:QC], func=F.Exp,
                            bias=km_t[:, i:i + 1], scale=SCALE)
                        if mask_future and i >= diag0:
                            nc.vector.tensor_mul(
                                pt[:, c0:c0 + PB], pt[:, c0:c0 + PB], cz_t)
                        nc.tensor.matmul(
                            cps[:, c0:QC], v_sb[i][:, h * (HD + 1):
                                                   (h + 1) * (HD + 1)],
                            pt[:, c0:QC],
                            start=(si == 0), stop=(si == len(order) - 1))
                    csb = ctxs_pool.tile([HD + 1, QC], f32, tag="c")
                    nc.vector.tensor_copy(csb, cps)
                    for r in range(QC // PB if _STAGE >= 3 else 0):
                        jt = j * (QC // PB) + r
                        tp = pp_t.tile([PB, HD + 1], f32, tag="t")
                        nc.tensor.matmul(
                            tp, csb[:, r * PB:(r + 1) * PB],
                            id_t[0:HD + 1, 0:HD + 1], is_transpose=True)
                        rc = rec_pool.tile([PB, 1], f32, tag="r")
                        nc.vector.reciprocal(rc, tp[:, HD:HD + 1])
                        nc.vector.tensor_mul(rc, rc, qm_t[:, jt:jt + 1])
                        nc.vector.tensor_scalar_mul(
                            out_sb[jt][:, h * HD:(h + 1) * HD],
                            tp[:, 0:HD], rc)
            for jt in range(NT if _STAGE >= 3 else 0):
                nc.sync.dma_start(out=out[jt * PB:(jt + 1) * PB, :],
                                  in_=out_sb[jt])

    nc.compile()
    return nc


def _get_nc(mask_future: bool, qk_bias: bool, v_bias: bool):
    key = (mask_future, qk_bias, v_bias, _STAGE)
    if key not in _CACHE:
        _CACHE[key] = _build(*key[:3])
    return _CACHE[key]


def _in_maps(query_states, key_states, query_mask, key_mask,
             Wq, bq, Wk, bk, Wv, bv, mask_future, qk_bias, v_bias):
    f4 = np.float32
    ident = np.eye(PB, dtype=f4)
    ones_g = np.ones((PB, HG), dtype=f4)
    causal = np.triu(np.ones((PB, PB), dtype=f4))
    in_maps = []
    for c in range(NCORES):
        b, g = c // BG, c % BG
        s = slice(g * HDG, (g + 1) * HDG)
        m = {
            "xqT": np.ascontiguousarray(query_states[b].T, dtype=f4),
            "xkT": np.ascontiguousarray(key_states[b].T, dtype=f4),
            "wqT": np.ascontiguousarray(Wq[s, :].T, dtype=f4),
            "wkT": np.ascontiguousarray(Wk[s, :].T, dtype=f4),
            "wvT": np.ascontiguousarray(Wv[s, :].T, dtype=f4),
            "kmb": np.ascontiguousarray(
                ((np.asarray(key_mask[b], f4) - 1.0) * 10000.0)
                .reshape(NT, PB).T),
            "qm": np.ascontiguousarray(
                np.asarray(query_mask[b], f4).reshape(NT, PB).T),
            "ident": ident,
            "ones_g": ones_g,
        }
        if mask_future:
            m["causal"] = causal
        if qk_bias:
            m["bq2"] = np.ascontiguousarray(
                np.asarray(bq[s], f4).reshape(2, PB).T)
            m["bk2"] = np.ascontiguousarray(
                np.asarray(bk[s], f4).reshape(2, PB).T)
        if v_bias:
            m["bvb"] = np.ascontiguousarray(
                np.broadcast_to(np.asarray(bv[s], f4), (PB, HDG)))
        in_maps.append(m)
    return in_maps


def kernel(query_states, key_states, query_mask, key_mask,
           Wq, bq, Wk, bk, Wv, bv, mask_future):
    query_states = np.asarray(query_states, np.float32)
    key_states = np.asarray(key_states, np.float32)
    mask_future = bool(int(np.asarray(mask_future)))
    qk_bias = bool(np.any(np.asarray(bq)) or np.any(np.asarray(bk)))
    v_bias = bool(np.any(np.asarray(bv)))

    nc = _get_nc(mask_future, qk_bias, v_bias)
    in_maps = _in_maps(query_states, key_states, query_mask, key_mask,
                       Wq, bq, Wk, bk, Wv, bv, mask_future, qk_bias, v_bias)
    res = run_bass_kernel_spmd(nc, in_maps, core_ids=list(range(NCORES)))
    full = np.empty((B, T, D), np.float32)
    for c in range(NCORES):
        b, g = c // BG, c % BG
        full[b][:, g * HDG:(g + 1) * HDG] = res.results[c]["out"]
    return full


# ---------------------------------------------------------------------------
# helpers for test.py (not used by the grader)

def timed_run(inputs, iters=10):
    """Run the kernel repeatedly through one jitted PJRT executable and
    return (first_results_full_output, list of per-iter wall seconds)."""
    import jax
    from jax.sharding import Mesh, PartitionSpec
    from jax.experimental.shard_map import shard_map
    from concourse import bass2jax

    mask_future = bool(int(np.asarray(inputs["mask_future"])))
    qk_bias = bool(np.any(np.asarray(inputs["bq"])) or
                   np.any(np.asarray(inputs["bk"])))
    v_bias = bool(np.any(np.asarray(inputs["bv"])))
    nc = _get_nc(mask_future, qk_bias, v_bias)
    in_maps = _in_maps(
        np.asarray(inputs["query_states"], np.float32),
        np.asarray(inputs["key_states"], np.float32),
        inputs["query_mask"], inputs["key_mask"],
        inputs["Wq"], inputs["bq"], inputs["Wk"], inputs["bk"],
        inputs["Wv"], inputs["bv"], mask_future, qk_bias, v_bias)

    bass2jax.install_neuronx_cc_hook()
    partition_name = (nc.partition_id_tensor.name
                      if nc.partition_id_tensor else None)
    in_names, out_names, out_avals, zero_outs = [], [], [], []
    for alloc in nc.m.functions[0].allocations:
        if not isinstance(alloc, mybir.MemoryLocationSet):
            continue
        name = alloc.memorylocations[0].name
        if alloc.kind == "ExternalInput":
            if name != partition_name:
                in_names.append(name)
        elif alloc.kind == "ExternalOutput":
            out_names.append(name)
            shape = tuple(alloc.tensor_shape)
            dtype = mybir.dt.np(alloc.dtype)
            out_avals.append(jax.core.ShapedArray(shape, dtype))
            zero_outs.append(np.zeros(shape, dtype))
    n_params = len(in_names)
    all_names = in_names + out_names
    if partition_name is not None:
        all_names.append(partition_name)

    def _body(*args):
        operands = list(args)
        if partition_name is not None:
            operands.append(bass2jax.partition_id_tensor())
        outs = bass2jax._bass_exec_p.bind(
            *operands, out_avals=tuple(out_avals), in_names=tuple(all_names),
            out_names=tuple(out_names), lowering_input_output_aliases=(),
            sim_require_finite=True, sim_require_nnan=True, nc=nc)
        return tuple(outs)

    devices = jax.devices()[:NCORES]
    mesh = Mesh(np.asarray(devices), ("core",))
    n_outs = len(out_names)
    sharded = jax.jit(
        shard_map(_body, mesh=mesh,
                  in_specs=(PartitionSpec("core"),) * (n_params + n_outs),
                  out_specs=(PartitionSpec("core"),) * n_outs,
                  check_rep=False),
        keep_unused=True)
    concat_in = [np.concatenate([np.asarray(in_maps[c][n]) for c in
                                 range(NCORES)], axis=0)
                 for n in in_names]
    concat_zeros = [np.zeros((NCORES * z.shape[0], *z.shape[1:]), z.dtype)
                    for z in zero_outs]
    dev_args = [jax.device_put(a) for a in concat_in + concat_zeros]
    outs = sharded(*dev_args)
    jax.block_until_ready(outs)
    times = []
    for _ in range(iters):
        t0 = time.perf_counter()
        outs = sharded(*dev_args)
        jax.block_until_ready(outs)
        times.append(time.perf_counter() - t0)
    full = np.empty((B, T, D), np.float32)
    arr = np.asarray(outs[out_names.index("out")]).reshape(NCORES, T, HDG)
    for c in range(NCORES):
        b, g = c // BG, c % BG
        full[b][:, g * HDG:(g + 1) * HDG] = arr[c]
    return full, times


def modeled_time_ns():
    """Cost-model (TimelineSim) estimate for the current cached module."""
    from concourse.timeline_sim import TimelineSim
    nc = next(iter(_CACHE.values()))
    return TimelineSim(nc, no_exec=True).simulate()
